# revision 1
# baseline (speedup 1.0000x reference)
"""GAT message-passing layer on 8 Trainium2 NeuronCores (Bass/Tile).

Strategy (matches the sharding hint): nodes are partitioned across the 8
cores; each edge is owned by the core that owns its destination node, so the
segment softmax and the weighted scatter-sum stay core-local.  Every core
computes the bf16 k/v projection table for all nodes (cheap, replicated),
plus q for its local nodes; per-edge k/v/q rows are then fetched with SWDGE
`dma_gather` (src-node "halo" reads), scores are reduced on the PE with a
block-diagonal head selector, the segment softmax numerator/denominator are
accumulated in PSUM via one-hot scatter matmuls, and the epilogue
(divide, residual, LN, FFN with PReLU folded into two weight matrices, LN)
runs per 128-node block.
"""

import sys

sys.path.insert(0, "/opt/trn_rl_repo")

import math
import os
from contextlib import ExitStack
from dataclasses import dataclass, field

import numpy as np
import ml_dtypes

import concourse.bass as bass
import concourse.bacc as bacc
import concourse.mybir as mybir
import concourse.tile as tile
from concourse._compat import with_exitstack
from concourse.bass_utils import run_bass_kernel_spmd
from concourse.library_config import mlp as mlp_lib

bf16 = ml_dtypes.bfloat16
P = 128
AF = mybir.ActivationFunctionType
OP = mybir.AluOpType
FP32 = mybir.dt.float32
BF16 = mybir.dt.bfloat16
I16 = mybir.dt.int16


@dataclass
class GATCfg:
    n_nodes: int = 50000
    n_edges: int = 640000
    feats: int = 128
    heads: int = 8
    dhead: int = 16
    dff: int = 512
    n_cores: int = 8
    grp: int = 2  # dst blocks per gather group
    wave: int = 4  # 128-edge subchunks per score/message wave
    tmult_chunk: int = 2048
    debug: bool = False

    @property
    def npc(self):  # nodes per core
        return self.n_nodes // self.n_cores

    @property
    def nblk(self):  # local 128-node blocks per core
        return (self.npc + P - 1) // P

    @property
    def local_pad(self):
        return self.nblk * P

    @property
    def npad(self):  # padded global node count (k/v table rows)
        return ((self.n_nodes + P - 1) // P) * P

    @property
    def half(self):  # int16 index split point (row offset base)
        h = self.npad // 2
        assert h < 32768 and (self.npad - h) <= 32768
        return h

    @property
    def ngrp(self):
        return (self.nblk + self.grp - 1) // self.grp


def _wrap16(idx):
    """int16 index list -> [128, n/16] SWDGE layout (16-wrap, replicated x8)."""
    idx = np.asarray(idx, np.int16)
    n = len(idx)
    assert n % 16 == 0
    return np.tile(idx.reshape(n // 16, 16).T, (8, 1)).copy()


def _prep(inputs, cfg: GATCfg):
    """Host-side graph partitioning / padding / index+S-matrix construction."""
    c = cfg
    feat = np.asarray(inputs["feat"], np.float32)
    src = np.asarray(inputs["src"], np.int64)
    dst = np.asarray(inputs["dst"], np.int64)

    feat_pad = np.zeros((c.npad, c.feats), np.float32)
    feat_pad[: c.n_nodes] = feat
    feat16 = feat_pad.astype(bf16)

    # ---- per (core, block, half) edge lists ----
    core_of = dst // c.npc
    per_core = []
    for ci in range(c.n_cores):
        sel = np.nonzero(core_of == ci)[0]
        dloc = dst[sel] - ci * c.npc
        blk = dloc // P
        half = (src[sel] >= c.half).astype(np.int64)
        order = np.lexsort((dloc, half, blk))
        sel, dloc, blk, half = sel[order], dloc[order], blk[order], half[order]
        lists = {}
        for b in range(c.nblk):
            for h in range(2):
                m = (blk == b) & (half == h)
                lists[(b, h)] = (src[sel[m]], dloc[m])
        per_core.append(lists)

    # uniform sub-chunk counts across cores
    n_sub = np.zeros((c.nblk, 2), np.int64)
    for b in range(c.nblk):
        for h in range(2):
            mx = max(len(per_core[ci][(b, h)][0]) for ci in range(c.n_cores))
            n_sub[b, h] = (mx + P - 1) // P

    # ---- group structure (shared across cores) ----
    groups = []  # list of dicts with static metadata
    scol = 0
    for g in range(c.ngrp):
        bs = list(range(g * c.grp, min((g + 1) * c.grp, c.nblk)))
        L_lo = int(sum(n_sub[b, 0] for b in bs)) * P
        L_hi = int(sum(n_sub[b, 1] for b in bs)) * P
        subs = []
        # per-block first/last sub bookkeeping (block's subs = its lo + hi subs)
        tot_per_block = {b: int(n_sub[b, 0] + n_sub[b, 1]) for b in bs}
        seen = {b: 0 for b in bs}
        s_idx = 0
        for h in range(2):
            for b in bs:
                for _ in range(int(n_sub[b, h])):
                    seen[b] += 1
                    subs.append(
                        dict(
                            block=b,
                            col=s_idx * P,
                            first=seen[b] == 1,
                            last=seen[b] == tot_per_block[b],
                        )
                    )
                    s_idx += 1
        groups.append(
            dict(bs=bs, L_lo=L_lo, L_hi=L_hi, L=L_lo + L_hi, subs=subs, scol=scol)
        )
        scol += L_lo + L_hi

    tot_cols = scol
    tot_lo = sum(g["L_lo"] for g in groups)
    tot_hi = sum(g["L_hi"] for g in groups)

    meta = dict(groups=groups, tot_cols=tot_cols, tot_lo=tot_lo, tot_hi=tot_hi)

    # ---- per-core streams ----
    per_core_streams = []
    for ci in range(c.n_cores):
        kv_lo = np.zeros(tot_lo, np.int16)
        kv_hi = np.zeros(tot_hi, np.int16)
        q_idx = np.zeros(tot_cols, np.int16)
        S = np.zeros((P, tot_cols), np.float32)
        olo = ohi = 0
        for g in groups:
            gcol = g["scol"]
            i = 0  # edge position within group tile
            for h in range(2):
                for b in g["bs"]:
                    s_arr, d_arr = per_core[ci][(b, h)]
                    npadded = int(n_sub[b, h]) * P
                    rel = np.zeros(npadded, np.int16)
                    rel[: len(s_arr)] = (s_arr - (c.half if h else 0)).astype(
                        np.int16
                    )
                    if h == 0:
                        kv_lo[olo : olo + npadded] = rel
                        olo += npadded
                    else:
                        kv_hi[ohi : ohi + npadded] = rel
                        ohi += npadded
                    dl = np.zeros(npadded, np.int16)
                    dl[: len(d_arr)] = d_arr.astype(np.int16)
                    q_idx[gcol + i : gcol + i + npadded] = dl
                    # one-hot S: edge j (pos i+j) -> col 128*s + (dloc - b*128)
                    jj = np.arange(len(d_arr))
                    pos = i + jj
                    ss = pos // P
                    pp = pos % P
                    S[pp, gcol + ss * P + (d_arr - b * P)] = 1.0
                    i += npadded
        feat32_loc = np.zeros((c.local_pad, c.feats), np.float32)
        feat32_loc[: c.npc] = feat[ci * c.npc : (ci + 1) * c.npc]
        feat16_loc = feat32_loc.astype(bf16)
        per_core_streams.append(
            dict(
                kv_idx_lo=_wrap16(kv_lo),
                kv_idx_hi=_wrap16(kv_hi),
                q_idx=_wrap16(q_idx),
                S_all=S.astype(bf16),
                feat32_loc=feat32_loc,
                feat16_loc=feat16_loc,
            )
        )

    # ---- shared weight/constant tensors ----
    W1 = np.asarray(inputs["W1"], np.float32)
    W2 = np.asarray(inputs["W2"], np.float32)
    a = np.asarray(inputs["prelu_a"], np.float32)
    # prelu(x) = max(x,0) + a*min(x,0) = ((1+a)/2)*x + ((1-a)/2)*|x|
    nh = c.dff // P
    # [dff, F] -> [P, nh, F] so each head-slice is an SBUF [128 x F] lhsT
    W2a = (
        (((1.0 + a) / 2.0)[:, None] * W2)
        .reshape(nh, P, c.feats)
        .transpose(1, 0, 2)
        .astype(bf16)
    )
    W2b = (
        (((1.0 - a) / 2.0)[:, None] * W2)
        .reshape(nh, P, c.feats)
        .transpose(1, 0, 2)
        .astype(bf16)
    )
    Rm = np.zeros((c.feats, c.heads), np.float32)
    Rm[np.arange(c.feats), np.arange(c.feats) // c.dhead] = 1.0
    shared = dict(
        feat16=feat16,
        wq=np.asarray(inputs["Wq"], np.float32).astype(bf16),
        wk=np.asarray(inputs["Wk"], np.float32).astype(bf16),
        wv=np.asarray(inputs["Wv"], np.float32).astype(bf16),
        w1=W1.astype(bf16),
        w2a=W2a,
        w2b=W2b,
        b1t=np.ascontiguousarray(
            np.asarray(inputs["b1"], np.float32).reshape(nh, P).T
        ),
        b2rep=np.tile(np.asarray(inputs["b2"], np.float32)[None, :], (P, 1)),
        grep=np.tile(np.asarray(inputs["ln1_g"], np.float32)[None, :], (P, 1)),
        brep=np.tile(np.asarray(inputs["ln1_b"], np.float32)[None, :], (P, 1)),
        Rm=Rm.astype(bf16),
        ident=np.eye(P, dtype=np.float32).astype(bf16),
    )
    return meta, per_core_streams, shared


@with_exitstack
def _emit(ctx: ExitStack, tc: tile.TileContext, t, meta, cfg: GATCfg):
    """Emit the per-core program. `t` maps tensor name -> DRAM AP."""
    c = cfg
    nc = tc.nc
    groups = meta["groups"]
    nh = c.dff // P
    scale = 1.0 / math.sqrt(c.heads * c.dhead)

    with tc.tile_critical():
        nc.gpsimd.load_library(mlp_lib)

    # ---------- persistent pool: constants, indices, ft2 storage ----------
    keep = ctx.enter_context(tc.tile_pool(name="keep", bufs=1))

    def load_const(name, shape, dtype):
        tl = keep.tile(shape, dtype, tag=name)
        nc.sync.dma_start(tl[:], t[name][:])
        return tl

    wq = load_const("wq", [P, P], BF16)
    wk = load_const("wk", [P, P], BF16)
    wv = load_const("wv", [P, P], BF16)
    w1 = load_const("w1", [P, c.dff], BF16)
    w2a = load_const("w2a", [P, nh, c.feats], BF16)
    w2b = load_const("w2b", [P, nh, c.feats], BF16)
    b1t = load_const("b1t", [P, nh], FP32)
    b2rep = load_const("b2rep", [P, P], FP32)
    grep = load_const("grep", [P, P], FP32)
    brep = load_const("brep", [P, P], FP32)
    Rm = load_const("Rm", [P, c.heads], BF16)
    ident = load_const("ident", [P, P], BF16)
    kvlo = load_const("kv_idx_lo", [P, max(meta["tot_lo"] // 16, 1)], I16)
    kvhi = load_const("kv_idx_hi", [P, max(meta["tot_hi"] // 16, 1)], I16)
    qidx = load_const("q_idx", [P, meta["tot_cols"] // 16], I16)

    ftden_sb = keep.tile([P, c.nblk, 136], FP32, tag="ftden_sb")

    dram = ctx.enter_context(tc.tile_pool(name="dram", bufs=1, space="DRAM"))
    kv_table = dram.tile([c.npad, 2 * c.feats], BF16)
    q_table = dram.tile([c.local_pad, c.feats], BF16)

    # ---------- phase 1: projection tables ----------
    with (
        tc.tile_pool(name="prj_ft", bufs=4) as prj_ft,
        tc.tile_pool(name="prj_ps", bufs=4, space="PSUM") as prj_ps,
        tc.tile_pool(name="prj_sb", bufs=4) as prj_sb,
    ):
        for blk in range(c.npad // P):
            ftT = prj_ft.tile([P, P], BF16, tag="ftT")
            nc.sync.dma_start(
                ftT[:], t["feat16"][blk * P : (blk + 1) * P, :], transpose=True
            )
            ps = prj_ps.tile([P, 2 * c.feats], FP32, tag="kvps")
            nc.tensor.matmul(ps[:, 0 : c.feats], ftT[:], wk[:], start=True, stop=True)
            nc.tensor.matmul(
                ps[:, c.feats : 2 * c.feats], ftT[:], wv[:], start=True, stop=True
            )
            sb = prj_sb.tile([P, 2 * c.feats], BF16, tag="kvsb")
            nc.any.tensor_copy(sb[:], ps[:])
            nc.sync.dma_start(kv_table[blk * P : (blk + 1) * P, :], sb[:])
        for blk in range(c.nblk):
            ftT = prj_ft.tile([P, P], BF16, tag="ftT")
            nc.sync.dma_start(
                ftT[:],
                t["feat16_loc"][blk * P : (blk + 1) * P, :],
                transpose=True,
            )
            ps = prj_ps.tile([P, c.feats], FP32, tag="qps")
            nc.tensor.matmul(ps[:], ftT[:], wq[:], start=True, stop=True)
            sb = prj_sb.tile([P, c.feats], BF16, tag="qsb")
            nc.any.tensor_copy(sb[:], ps[:])
            nc.sync.dma_start(q_table[blk * P : (blk + 1) * P, :], sb[:])

    GCHUNK = 1 << 30  # single big gather per (group, half); needs single_packet=False

    def gather_chunked(out_fn, in_ap, idx_tile, idx_off, n, elem, estep, transpose, qn=[0]):
        """Emit dma_gather in <=512-index chunks (HW ring limit), round-robin queues."""
        for o in range(0, n, GCHUNK):
            m = min(GCHUNK, n - o)
            nc.gpsimd.dma_gather(
                out_fn(o, m),
                in_ap,
                idx_tile[:, (idx_off + o) // 16 : (idx_off + o + m) // 16],
                m,
                m,
                elem,
                elem_step=estep,
                transpose=transpose,
                single_packet=False,
            )

    # ---------- phase 2: edge sweep ----------
    # hard barrier: the projection phase's DMA-transpose loads must not
    # overlap the transpose-mode dma_gathers (xbar-mode HW hazard that Tile
    # does not track for InstDMAGatherAnt)
    if os.environ.get("GAT_PHASE_BARRIER") != "0":
        tc.strict_bb_all_engine_barrier()
    lmax = max(g["L"] for g in groups)
    with (
        tc.tile_pool(name="eg_gather", bufs=3) as eg_gather,
        tc.tile_pool(name="eg_tt", bufs=2) as eg_tt,
        tc.tile_pool(name="eg_wave", bufs=3) as eg_wave,
        tc.tile_pool(name="eg_scps", bufs=2, space="PSUM") as eg_scps,
        tc.tile_pool(name="eg_ftps", bufs=c.grp + 1, space="PSUM") as eg_ftps,
    ):
        olo = ohi = 0
        for g in groups:
            L, L_lo, L_hi = g["L"], g["L_lo"], g["L_hi"]
            kT = eg_gather.tile([P, 1, lmax], BF16, tag="kT")
            qT = eg_gather.tile([P, 1, lmax], BF16, tag="qT")
            vE = eg_gather.tile([P, lmax // P, P], BF16, tag="vE")
            Ssb = eg_gather.tile([P, lmax], BF16, tag="Ssb")
            nc.sync.dma_start(
                Ssb[:, 0:L], t["S_all"][:, g["scol"] : g["scol"] + L]
            )
            no_gather = os.environ.get("GAT_NO_GATHER") == "1"
            if no_gather:
                nc.vector.memset(kT[:, :, 0:L], 0.25)
                nc.vector.memset(qT[:, :, 0:L], 0.25)
                nc.vector.memset(vE[:, 0 : L // P, :], 0.25)
            if L_lo and not no_gather:
                gather_chunked(
                    lambda o, m: kT[:, :, o : o + m],
                    kv_table[:][0 : c.half, 0 : c.feats],
                    kvlo, olo, L_lo, c.feats, 2 * c.feats, True,
                )
                gather_chunked(
                    lambda o, m: vE[:, o // P : (o + m) // P, :],
                    kv_table[:][0 : c.half, c.feats : 2 * c.feats],
                    kvlo, olo, L_lo, c.feats, 2 * c.feats, False,
                )
            if L_hi and not no_gather:
                gather_chunked(
                    lambda o, m: kT[:, :, L_lo + o : L_lo + o + m],
                    kv_table[:][c.half : c.npad, 0 : c.feats],
                    kvhi, ohi, L_hi, c.feats, 2 * c.feats, True,
                )
                gather_chunked(
                    lambda o, m: vE[:, (L_lo + o) // P : (L_lo + o + m) // P, :],
                    kv_table[:][c.half : c.npad, c.feats : 2 * c.feats],
                    kvhi, ohi, L_hi, c.feats, 2 * c.feats, False,
                )
            if not no_gather:
                gather_chunked(
                    lambda o, m: qT[:, :, o : o + m],
                    q_table[:],
                    qidx, g["scol"], L, c.feats, None, True,
                )
            olo += L_lo
            ohi += L_hi

            TT = eg_tt.tile([P, lmax], BF16, tag="TT")
            for off in range(0, L, c.tmult_chunk):
                w = min(c.tmult_chunk, L - off)
                nc.vector.tensor_tensor(
                    TT[:, off : off + w],
                    kT[:, 0, off : off + w],
                    qT[:, 0, off : off + w],
                    op=OP.mult,
                )

            ftps = {}
            subs = g["subs"]
            for w0 in range(0, len(subs), c.wave):
                wsubs = subs[w0 : w0 + c.wave]
                wl = len(wsubs)
                scps = eg_scps.tile([P, c.wave * c.heads], FP32, tag="scps")
                for i, s in enumerate(wsubs):
                    nc.tensor.matmul(
                        scps[:, i * c.heads : (i + 1) * c.heads],
                        TT[:, s["col"] : s["col"] + P],
                        Rm[:],
                        start=True,
                        stop=True,
                    )
                pexp = eg_wave.tile([P, c.wave, c.heads, c.dhead], BF16, tag="pexp")
                nc.scalar.activation(
                    pexp[:, 0:wl],
                    scps[:, 0 : wl * c.heads]
                    .rearrange("p (a h) -> p a h", h=c.heads)
                    .to_broadcast([P, wl, c.heads, c.dhead]),
                    AF.Exp,
                    scale=scale,
                )
                Mt = eg_wave.tile([P, c.wave, P], BF16, tag="Mt")
                nc.vector.tensor_tensor(
                    Mt[:, 0:wl],
                    vE[:, w0 : w0 + wl, :],
                    pexp[:, 0:wl].rearrange("p a h d -> p a (h d)"),
                    op=OP.mult,
                )
                for i, s in enumerate(wsubs):
                    b = s["block"]
                    if s["first"]:
                        ftps[b] = eg_ftps.tile(
                            [P, 136], FP32, tag="ftps", name="ftps"
                        )
                    nc.tensor.matmul(
                        ftps[b][:, 0:128],
                        Ssb[:, s["col"] : s["col"] + P],
                        Mt[:, i, :],
                        start=s["first"],
                        stop=s["last"],
                        skip_group_check=True,
                    )
                    # ft2's start already marked this bank pending-zero, so
                    # the first denom write lands on zeroed bytes with start=False
                    if os.environ.get("GAT_NO_DENOM") != "1":
                        nc.tensor.matmul(
                            ftps[b][:, 128:136],
                            Ssb[:, s["col"] : s["col"] + P],
                            pexp[:, i, :, 0] if os.environ.get("GAT_DEN_CONTIG") != "1" else pexp[:, i, 0, 0:8],
                            start=False,
                            stop=s["last"],
                            skip_group_check=True,
                        )
                    elif s["last"]:
                        nc.vector.memset(ftps[b][:, 128:136], 1.0)
                    if s["last"]:
                        nc.any.tensor_copy(ftden_sb[:, b, :], ftps[b][:])

    # ---------- phase 3: epilogue (divide, residual, LN1, FFN, LN2) ----------
    def layernorm(pool, x32, nb, out_dtype=FP32):
        """x32: [P, nb, 128] fp32 SBUF tile -> normalized * g + b (new tile)."""
        msum = pool.tile([P, c.grp], FP32, tag="ln_msum")
        nc.vector.tensor_reduce(
            msum[:, 0:nb], x32[:, 0:nb, :], axis=mybir.AxisListType.X, op=OP.add
        )
        nmean = pool.tile([P, c.grp], FP32, tag="ln_nmean")
        nc.vector.tensor_scalar_mul(nmean[:, 0:nb], msum[:, 0:nb], -1.0 / c.feats)
        sq = pool.tile([P, c.grp, P], FP32, tag="ln_sq")
        for b in range(nb):
            nc.scalar.activation(
                sq[:, b],
                x32[:, b],
                AF.Square,
                bias=nmean[:, b : b + 1],
            )
        var = pool.tile([P, c.grp], FP32, tag="ln_var")
        nc.vector.tensor_reduce(
            var[:, 0:nb], sq[:, 0:nb, :], axis=mybir.AxisListType.X, op=OP.add
        )
        rstd = pool.tile([P, c.grp], FP32, tag="ln_rstd")
        nc.vector.tensor_scalar(
            rstd[:, 0:nb], var[:, 0:nb], 1.0 / c.feats, 1e-5, op0=OP.mult, op1=OP.add
        )
        nc.vector.reciprocal(rstd[:, 0:nb], rstd[:, 0:nb])
        nc.scalar.sqrt(rstd[:, 0:nb], rstd[:, 0:nb])
        nmr = pool.tile([P, c.grp], FP32, tag="ln_nmr")
        nc.vector.tensor_tensor(
            nmr[:, 0:nb], nmean[:, 0:nb], rstd[:, 0:nb], op=OP.mult
        )
        normed = pool.tile([P, c.grp, P], FP32, tag="ln_normed")
        for b in range(nb):
            nc.scalar.activation(
                normed[:, b],
                x32[:, b],
                AF.Identity,
                scale=rstd[:, b : b + 1],
                bias=nmr[:, b : b + 1],
            )
        out = pool.tile([P, c.grp, P], out_dtype, tag="ln_out" + str(out_dtype))
        nc.vector.tensor_tensor(
            out[:, 0:nb],
            normed[:, 0:nb],
            grep[:].rearrange("p (o f) -> p o f", o=1).to_broadcast([P, nb, P]),
            op=OP.mult,
        )
        nc.vector.tensor_tensor(
            out[:, 0:nb],
            out[:, 0:nb],
            brep[:].rearrange("p (o f) -> p o f", o=1).to_broadcast([P, nb, P]),
            op=OP.add,
        )
        return out

    if c.debug:
        nc.sync.dma_start(t["dbg_ftden"][:], ftden_sb[:])
    with (
        tc.tile_pool(name="ep", bufs=2) as ep,
        tc.tile_pool(name="ep_ps", bufs=2, space="PSUM") as ep_ps,
        tc.tile_pool(name="ep_h1ps", bufs=2, space="PSUM") as ep_h1ps,
    ):
        for g in groups:
            bs = g["bs"]
            nb = len(bs)
            b0 = bs[0]
            f32 = ep.tile([P, c.grp, P], FP32, tag="f32")
            nc.sync.dma_start(
                f32[:, 0:nb, :],
                t["feat32_loc"][:]
                .rearrange("(s p) f -> p s f", p=P)[:, b0 : b0 + nb, :],
            )
            r = ep.tile([P, c.grp, c.heads], FP32, tag="recip")
            nc.vector.tensor_scalar_max(
                r[:, 0:nb], ftden_sb[:, b0 : b0 + nb, 128:136], 1e-30
            )
            nc.vector.reciprocal(r[:, 0:nb], r[:, 0:nb])
            rst = ep.tile([P, c.grp, P], FP32, tag="rst")
            nc.vector.tensor_tensor(
                rst[:, 0:nb],
                ftden_sb[:, b0 : b0 + nb, 0:128].rearrange(
                    "p s (h d) -> p s h d", d=c.dhead
                ),
                r[:, 0:nb].rearrange("p s (h o) -> p s h o", o=1).to_broadcast(
                    [P, nb, c.heads, c.dhead]
                ),
                op=OP.mult,
            )
            nc.vector.tensor_tensor(
                rst[:, 0:nb], rst[:, 0:nb], f32[:, 0:nb, :], op=OP.add
            )
            ln1 = layernorm(ep, rst, nb)
            ln1b = ep.tile([P, c.grp, P], BF16, tag="ln1b")
            nc.scalar.copy(ln1b[:, 0:nb], ln1[:, 0:nb])
            # transpose ln1 -> feat-major for FFN
            rT_ps = ep_ps.tile([P, c.grp * P], BF16, tag="rT_ps")
            for b in range(nb):
                nc.tensor.transpose(
                    rT_ps[:, b * P : (b + 1) * P], ln1b[:, b, :], ident[:]
                )
            rT = ep.tile([P, c.grp * P], BF16, tag="rT")
            nc.vector.tensor_copy(rT[:, 0 : nb * P], rT_ps[:, 0 : nb * P])
            # H1 = W1.T @ rT  (feat-major, nh slices) ; prelu via W2a/W2b trick
            ffps = ep_ps.tile([P, c.grp * P], FP32, tag="ffps")
            for h in range(nh):
                h1ps = ep_h1ps.tile([P, c.grp * P], FP32, tag="h1ps")
                nc.tensor.matmul(
                    h1ps[:, 0 : nb * P],
                    w1[:, h * P : (h + 1) * P],
                    rT[:, 0 : nb * P],
                    start=True,
                    stop=True,
                )
                h1sb = ep.tile([P, c.grp * P], BF16, tag="h1sb")
                nc.scalar.activation(
                    h1sb[:, 0 : nb * P],
                    h1ps[:, 0 : nb * P],
                    AF.Identity,
                    bias=b1t[:, h : h + 1],
                )
                habs = ep.tile([P, c.grp * P], BF16, tag="habs")
                nc.scalar.activation(
                    habs[:, 0 : nb * P],
                    h1ps[:, 0 : nb * P],
                    AF.Abs,
                    bias=b1t[:, h : h + 1],
                )
                for b in range(nb):
                    nc.tensor.matmul(
                        ffps[:, b * P : (b + 1) * P],
                        h1sb[:, b * P : (b + 1) * P],
                        w2a[:, h, :],
                        start=(h == 0 and b == 0),
                        stop=False,
                        skip_group_check=True,
                    )
                    nc.tensor.matmul(
                        ffps[:, b * P : (b + 1) * P],
                        habs[:, b * P : (b + 1) * P],
                        w2b[:, h, :],
                        start=False,
                        stop=(h == nh - 1),
                        skip_group_check=True,
                    )
            rst2 = ep.tile([P, c.grp, P], FP32, tag="rst2")
            nc.vector.tensor_tensor(
                rst2[:, 0:nb],
                ffps[:, 0 : nb * P].rearrange("p (s f) -> p s f", f=P),
                ln1[:, 0:nb],
                op=OP.add,
            )
            nc.vector.tensor_tensor(
                rst2[:, 0:nb],
                rst2[:, 0:nb],
                b2rep[:].rearrange("p (o f) -> p o f", o=1).to_broadcast([P, nb, P]),
                op=OP.add,
            )
            ln2 = layernorm(ep, rst2, nb)
            nc.sync.dma_start(
                t["out"][:].rearrange("(s p) f -> p s f", p=P)[:, b0 : b0 + nb, :],
                ln2[:, 0:nb],
            )


def _build(meta, cfg: GATCfg):
    c = cfg
    nc = bacc.Bacc("TRN2", target_bir_lowering=False, debug=False, num_devices=c.n_cores)
    t = {}

    def inp(name, shape, dtype):
        t[name] = nc.dram_tensor(name, shape, dtype, kind="ExternalInput").ap()

    inp("feat16", [c.npad, c.feats], BF16)
    inp("feat16_loc", [c.local_pad, c.feats], BF16)
    inp("feat32_loc", [c.local_pad, c.feats], FP32)
    inp("wq", [c.feats, c.feats], BF16)
    inp("wk", [c.feats, c.feats], BF16)
    inp("wv", [c.feats, c.feats], BF16)
    inp("w1", [c.feats, c.dff], BF16)
    inp("w2a", [P, c.dff // P, c.feats], BF16)
    inp("w2b", [P, c.dff // P, c.feats], BF16)
    inp("b1t", [P, c.dff // P], FP32)
    inp("b2rep", [P, c.feats], FP32)
    inp("grep", [P, c.feats], FP32)
    inp("brep", [P, c.feats], FP32)
    inp("Rm", [c.feats, c.heads], BF16)
    inp("ident", [P, P], BF16)
    inp("kv_idx_lo", [P, max(meta["tot_lo"] // 16, 1)], I16)
    inp("kv_idx_hi", [P, max(meta["tot_hi"] // 16, 1)], I16)
    inp("q_idx", [P, meta["tot_cols"] // 16], I16)
    inp("S_all", [P, meta["tot_cols"]], BF16)
    t["out"] = nc.dram_tensor(
        "out", [c.local_pad, c.feats], FP32, kind="ExternalOutput"
    ).ap()
    if c.debug:
        t["dbg_ftden"] = nc.dram_tensor(
            "dbg_ftden", [P, c.nblk, 136], FP32, kind="ExternalOutput"
        ).ap()

    with tile.TileContext(nc) as tc:
        _emit(tc, t, meta, cfg)
    nc.compile()
    return nc


def _in_maps(meta, streams, shared, cfg: GATCfg):
    maps = []
    for ci in range(cfg.n_cores):
        m = dict(shared)
        st = streams[ci]
        m["feat16_loc"] = st["feat16_loc"]
        m["feat32_loc"] = st["feat32_loc"]
        m["kv_idx_lo"] = (
            st["kv_idx_lo"]
            if meta["tot_lo"]
            else np.zeros((P, 1), np.int16)
        )
        m["kv_idx_hi"] = (
            st["kv_idx_hi"]
            if meta["tot_hi"]
            else np.zeros((P, 1), np.int16)
        )
        m["q_idx"] = st["q_idx"]
        m["S_all"] = st["S_all"]
        maps.append(m)
    return maps


_CACHE = {}


def kernel(**inputs) -> np.ndarray:
    cfg = GATCfg()
    meta, streams, shared = _prep(inputs, cfg)
    key = "real"
    if key not in _CACHE:
        _CACHE[key] = _build(meta, cfg)
    nc = _CACHE[key]
    maps = _in_maps(meta, streams, shared, cfg)
    res = run_bass_kernel_spmd(nc, maps, core_ids=list(range(cfg.n_cores)))
    out = np.empty((cfg.n_nodes, cfg.feats), np.float32)
    for ci in range(cfg.n_cores):
        out[ci * cfg.npc : (ci + 1) * cfg.npc] = res.results[ci]["out"][: cfg.npc]
    return out



# revision 15
# speedup vs baseline: 2.0527x; 2.0527x over previous
"""GAT message-passing layer on 8 Trainium2 NeuronCores (Bass/Tile).

Strategy (matches the sharding hint): nodes are partitioned across the 8
cores; each edge is owned by the core that owns its destination node, so the
segment softmax and the weighted scatter-sum stay core-local.  Every core
computes the bf16 k/v projection table for all nodes (cheap, replicated;
natural-layout loads + PE transposes — no slow DMA-transpose) and keeps q for
its local nodes in SBUF.  Per-edge k rows are fetched feat-major with SWDGE
`dma_gather` (transpose mode), v rows edge-major (row mode); per-edge q is NOT
gathered — within a sub all 128 edges share one dst block, so qT per edge is a
one-hot select matmul q_blk^T @ ST against the streamed ST matrix.  Scores are
reduced on the PE with a block-diagonal head selector, the segment softmax
numerator/denominator are accumulated in PSUM via one-hot scatter matmuls, and
the epilogue (divide, residual, LN, FFN with PReLU folded into two weight
matrices, LN) runs per 128-node block.
"""

import sys

sys.path.insert(0, "/opt/trn_rl_repo")

import math
import os
from contextlib import ExitStack
from dataclasses import dataclass, field

import numpy as np
import ml_dtypes

import concourse.bass as bass
import concourse.bacc as bacc
import concourse.mybir as mybir
import concourse.tile as tile
from concourse._compat import with_exitstack
from concourse.bass_utils import run_bass_kernel_spmd
from concourse.library_config import mlp as mlp_lib

bf16 = ml_dtypes.bfloat16
P = 128
AF = mybir.ActivationFunctionType
OP = mybir.AluOpType
FP32 = mybir.dt.float32
BF16 = mybir.dt.bfloat16
I16 = mybir.dt.int16


@dataclass
class GATCfg:
    n_nodes: int = 50000
    n_edges: int = 640000
    feats: int = 128
    heads: int = 8
    dhead: int = 16
    dff: int = 512
    n_cores: int = 8
    grp: int = 2  # dst blocks per gather group
    wave: int = 4  # 128-edge subchunks per score/message wave
    tmult_chunk: int = 2048
    qsel_chunk: int = 512  # qT select matmul column chunk (1 PSUM bank)
    debug: bool = False

    @property
    def npc(self):  # nodes per core
        return self.n_nodes // self.n_cores

    @property
    def nblk(self):  # local 128-node blocks per core
        return (self.npc + P - 1) // P

    @property
    def local_pad(self):
        return self.nblk * P

    @property
    def npad(self):  # padded global node count (k/v table rows)
        return ((self.n_nodes + P - 1) // P) * P

    @property
    def half(self):  # int16 index split point (row offset base)
        h = self.npad // 2
        assert h < 32768 and (self.npad - h) <= 32768
        return h

    @property
    def ngrp(self):
        return (self.nblk + self.grp - 1) // self.grp


def _wrap16(idx):
    """int16 index list -> [128, n/16] SWDGE layout (16-wrap, replicated x8)."""
    idx = np.asarray(idx, np.int16)
    n = len(idx)
    assert n % 16 == 0
    return np.tile(idx.reshape(n // 16, 16).T, (8, 1)).copy()


def _prep(inputs, cfg: GATCfg):
    """Host-side graph partitioning / padding / index+S-matrix construction."""
    c = cfg
    feat = np.asarray(inputs["feat"], np.float32)
    src = np.asarray(inputs["src"], np.int64)
    dst = np.asarray(inputs["dst"], np.int64)

    feat_pad = np.zeros((c.npad, c.feats), np.float32)
    feat_pad[: c.n_nodes] = feat
    feat16 = feat_pad.astype(bf16)

    # ---- per (core, block, half) edge lists ----
    core_of = dst // c.npc
    per_core = []
    for ci in range(c.n_cores):
        sel = np.nonzero(core_of == ci)[0]
        dloc = dst[sel] - ci * c.npc
        blk = dloc // P
        half = (src[sel] >= c.half).astype(np.int64)
        order = np.lexsort((dloc, half, blk))
        sel, dloc, blk, half = sel[order], dloc[order], blk[order], half[order]
        lists = {}
        for b in range(c.nblk):
            for h in range(2):
                m = (blk == b) & (half == h)
                lists[(b, h)] = (src[sel[m]], dloc[m])
        per_core.append(lists)

    # uniform sub-chunk counts across cores
    n_sub = np.zeros((c.nblk, 2), np.int64)
    for b in range(c.nblk):
        for h in range(2):
            mx = max(len(per_core[ci][(b, h)][0]) for ci in range(c.n_cores))
            n_sub[b, h] = (mx + P - 1) // P

    # ---- group structure (shared across cores) ----
    groups = []  # list of dicts with static metadata
    scol = 0
    for g in range(c.ngrp):
        bs = list(range(g * c.grp, min((g + 1) * c.grp, c.nblk)))
        L_lo = int(sum(n_sub[b, 0] for b in bs)) * P
        L_hi = int(sum(n_sub[b, 1] for b in bs)) * P
        subs = []
        runs = []  # contiguous (block, col, ncols) spans for the qT select
        # per-block first/last sub bookkeeping (block's subs = its lo + hi subs)
        tot_per_block = {b: int(n_sub[b, 0] + n_sub[b, 1]) for b in bs}
        seen = {b: 0 for b in bs}
        s_idx = 0
        for h in range(2):
            for b in bs:
                ns = int(n_sub[b, h])
                if ns:
                    runs.append(dict(block=b, col=s_idx * P, ncols=ns * P))
                for _ in range(ns):
                    seen[b] += 1
                    subs.append(
                        dict(
                            block=b,
                            col=s_idx * P,
                            first=seen[b] == 1,
                            last=seen[b] == tot_per_block[b],
                        )
                    )
                    s_idx += 1
        groups.append(
            dict(
                bs=bs, L_lo=L_lo, L_hi=L_hi, L=L_lo + L_hi, subs=subs,
                runs=runs, scol=scol,
            )
        )
        scol += L_lo + L_hi

    tot_cols = scol
    tot_lo = sum(g["L_lo"] for g in groups)
    tot_hi = sum(g["L_hi"] for g in groups)

    meta = dict(groups=groups, tot_cols=tot_cols, tot_lo=tot_lo, tot_hi=tot_hi)

    # ---- per-core streams ----
    per_core_streams = []
    for ci in range(c.n_cores):
        kv_lo = np.zeros(tot_lo, np.int16)
        kv_hi = np.zeros(tot_hi, np.int16)
        S = np.zeros((P, tot_cols), np.float32)
        ST = np.zeros((P, tot_cols), np.float32)
        olo = ohi = 0
        for g in groups:
            gcol = g["scol"]
            i = 0  # edge position within group tile
            for h in range(2):
                for b in g["bs"]:
                    s_arr, d_arr = per_core[ci][(b, h)]
                    npadded = int(n_sub[b, h]) * P
                    rel = np.zeros(npadded, np.int16)
                    rel[: len(s_arr)] = (s_arr - (c.half if h else 0)).astype(
                        np.int16
                    )
                    if h == 0:
                        kv_lo[olo : olo + npadded] = rel
                        olo += npadded
                    else:
                        kv_hi[ohi : ohi + npadded] = rel
                        ohi += npadded
                    # one-hot S: edge j (pos i+j) -> col 128*s + (dloc - b*128)
                    jj = np.arange(len(d_arr))
                    pos = i + jj
                    ss = pos // P
                    pp = pos % P
                    S[pp, gcol + ss * P + (d_arr - b * P)] = 1.0
                    # one-hot ST (transposed layout): row = dst slot, col = pos
                    ST[d_arr - b * P, gcol + pos] = 1.0
                    i += npadded
        feat32_loc = np.zeros((c.local_pad, c.feats), np.float32)
        feat32_loc[: c.npc] = feat[ci * c.npc : (ci + 1) * c.npc]
        per_core_streams.append(
            dict(
                kv_idx_lo=_wrap16(kv_lo),
                kv_idx_hi=_wrap16(kv_hi),
                S_all=S.astype(bf16),
                ST_all=ST.astype(bf16),
                feat32_loc=feat32_loc,
            )
        )

    # ---- shared weight/constant tensors ----
    W1 = np.asarray(inputs["W1"], np.float32)
    W2 = np.asarray(inputs["W2"], np.float32)
    a = np.asarray(inputs["prelu_a"], np.float32)
    # prelu(x) = max(x,0) + a*min(x,0) = ((1+a)/2)*x + ((1-a)/2)*|x|
    nh = c.dff // P
    # [dff, F] -> [P, nh, F] so each head-slice is an SBUF [128 x F] lhsT
    W2a = (
        (((1.0 + a) / 2.0)[:, None] * W2)
        .reshape(nh, P, c.feats)
        .transpose(1, 0, 2)
        .astype(bf16)
    )
    W2b = (
        (((1.0 - a) / 2.0)[:, None] * W2)
        .reshape(nh, P, c.feats)
        .transpose(1, 0, 2)
        .astype(bf16)
    )
    Rm = np.zeros((c.feats, c.heads), np.float32)
    Rm[np.arange(c.feats), np.arange(c.feats) // c.dhead] = 1.0
    wkv = np.concatenate(
        [np.asarray(inputs["Wk"], np.float32), np.asarray(inputs["Wv"], np.float32)],
        axis=1,
    )
    shared = dict(
        feat16=feat16,
        wq=np.asarray(inputs["Wq"], np.float32).astype(bf16),
        wkv=wkv.astype(bf16),
        w1=W1.astype(bf16),
        w2a=W2a,
        w2b=W2b,
        b1t=np.ascontiguousarray(
            np.asarray(inputs["b1"], np.float32).reshape(nh, P).T
        ),
        b2rep=np.tile(np.asarray(inputs["b2"], np.float32)[None, :], (P, 1)),
        grep=np.tile(np.asarray(inputs["ln1_g"], np.float32)[None, :], (P, 1)),
        brep=np.tile(np.asarray(inputs["ln1_b"], np.float32)[None, :], (P, 1)),
        Rm=Rm.astype(bf16),
        ident=np.eye(P, dtype=np.float32).astype(bf16),
    )
    return meta, per_core_streams, shared


@with_exitstack
def _emit(ctx: ExitStack, tc: tile.TileContext, t, meta, cfg: GATCfg):
    """Emit the per-core program. `t` maps tensor name -> DRAM AP."""
    c = cfg
    nc = tc.nc
    groups = meta["groups"]
    nh = c.dff // P
    scale = 1.0 / math.sqrt(c.heads * c.dhead)

    with tc.tile_critical():
        nc.gpsimd.load_library(mlp_lib)

    # ---------- persistent pool: constants, indices, q + ft2 storage ----------
    keep = ctx.enter_context(tc.tile_pool(name="keep", bufs=1))

    def load_const(name, shape, dtype):
        tl = keep.tile(shape, dtype, tag=name)
        nc.sync.dma_start(tl[:], t[name][:])
        return tl

    wq = load_const("wq", [P, P], BF16)
    wkv = load_const("wkv", [P, 2 * c.feats], BF16)
    w1 = load_const("w1", [P, c.dff], BF16)
    w2a = load_const("w2a", [P, nh, c.feats], BF16)
    w2b = load_const("w2b", [P, nh, c.feats], BF16)
    b1t = load_const("b1t", [P, nh], FP32)
    b2rep = load_const("b2rep", [P, P], FP32)
    grep = load_const("grep", [P, P], FP32)
    brep = load_const("brep", [P, P], FP32)
    Rm = load_const("Rm", [P, c.heads], BF16)
    ident = load_const("ident", [P, P], BF16)
    kvlo = load_const("kv_idx_lo", [P, max(meta["tot_lo"] // 16, 1)], I16)
    kvhi = load_const("kv_idx_hi", [P, max(meta["tot_hi"] // 16, 1)], I16)

    q_sb = keep.tile([P, c.nblk, c.feats], BF16, tag="q_sb")
    ftden_sb = keep.tile([P, c.nblk, 136], FP32, tag="ftden_sb")

    dram = ctx.enter_context(tc.tile_pool(name="dram", bufs=1, space="DRAM"))
    kv_table = dram.tile([c.npad, 2 * c.feats], BF16)

    # ---------- phase 1: projection tables ----------
    # natural-layout loads + PE transpose (DMA-transpose is ~4us/tile serialized)
    feat_r = t["feat16"][:].rearrange("(s p) f -> p s f", p=P)
    kv_r = kv_table[:].rearrange("(s p) f -> p s f", p=P)
    PB = 2  # blocks per DMA batch
    with (
        tc.tile_pool(name="prj_ft", bufs=3) as prj_ft,
        tc.tile_pool(name="prj_tps", bufs=3, space="PSUM") as prj_tps,
        tc.tile_pool(name="prj_tsb", bufs=4) as prj_tsb,
        tc.tile_pool(name="prj_ps", bufs=3, space="PSUM") as prj_ps,
        tc.tile_pool(name="prj_sb", bufs=3) as prj_sb,
    ):
        nbl = c.npad // P
        n_kv_stores = (nbl + PB - 1) // PB
        for pair in range(n_kv_stores):
            blks = range(pair * PB, min((pair + 1) * PB, nbl))
            nb_ = len(blks)
            ft = prj_ft.tile([P, PB, P], BF16, tag="ft")
            nc.sync.dma_start(
                ft[:, 0:nb_, :], feat_r[:, pair * PB : pair * PB + nb_, :]
            )
            sb = prj_sb.tile([P, PB, 2 * c.feats], BF16, tag="kvsb")
            for s in range(nb_):
                tps = prj_tps.tile([P, P], BF16, tag="tps")
                nc.tensor.transpose(tps[:], ft[:, s, :], ident[:])
                tsb = prj_tsb.tile([P, P], BF16, tag="tsb")
                nc.any.tensor_copy(tsb[:], tps[:])
                ps = prj_ps.tile([P, 2 * c.feats], FP32, tag="kvps")
                nc.tensor.matmul(ps[:], tsb[:], wkv[:], start=True, stop=True)
                nc.any.tensor_copy(sb[:, s, :], ps[:])
            nc.sync.dma_start(
                kv_r[:, pair * PB : pair * PB + nb_, :], sb[:, 0:nb_, :]
            )
        # local q -> SBUF only (no DRAM round-trip)
        floc_r = t["feat16_loc"][:].rearrange("(s p) f -> p s f", p=P)
        for pair in range((c.nblk + PB - 1) // PB):
            blks = range(pair * PB, min((pair + 1) * PB, c.nblk))
            ft = prj_ft.tile([P, PB, P], BF16, tag="ft")
            nc.sync.dma_start(
                ft[:, 0 : len(blks), :], floc_r[:, pair * PB : pair * PB + len(blks), :]
            )
            for s, blk in enumerate(blks):
                tps = prj_tps.tile([P, P], BF16, tag="tps")
                nc.tensor.transpose(tps[:], ft[:, s, :], ident[:])
                tsb = prj_tsb.tile([P, P], BF16, tag="tsb")
                nc.any.tensor_copy(tsb[:], tps[:])
                ps = prj_ps.tile([P, 2 * c.feats], FP32, tag="kvps")
                nc.tensor.matmul(
                    ps[:, 0 : c.feats], tsb[:], wq[:], start=True, stop=True
                )
                nc.any.tensor_copy(q_sb[:, blk, :], ps[:, 0 : c.feats])

    GCHUNK = 1 << 30  # single big gather per (group, half); needs single_packet=False

    def gather_chunked(out_fn, in_ap, idx_tile, idx_off, n, elem, estep, transpose):
        for o in range(0, n, GCHUNK):
            m = min(GCHUNK, n - o)
            nc.gpsimd.dma_gather(
                out_fn(o, m),
                in_ap,
                idx_tile[:, (idx_off + o) // 16 : (idx_off + o + m) // 16],
                m,
                m,
                elem,
                elem_step=estep,
                transpose=transpose,
                single_packet=False,
            )

    # ---------- phase 2: edge sweep ----------
    # hard barrier: phase 1's kv_table stores must complete before the gathers
    # read them (Tile does not track DRAM deps for InstDMAGatherAnt). The
    # barrier alone only orders instruction *issue* — the store DMAs are still
    # in flight — so gate the gather engine on the stores' completion sems.
    # Phase-1 kv stores -> gather reads: Tile's wait pass covers the stores'
    # DMA completions only transitively through the sb-tile WAR reuse chain,
    # which misses the LAST ~bufs store pairs (nobody ever waits their DMAHW
    # sems; the gathers get only an engine-retire wait). Guard: read back the
    # tail of kv_table with a regular load (RAW => real DMAHW wait), and make
    # the gather engine consume it so every gather orders after it.
    nblk_all = c.npad // P
    TAILB = min(nblk_all, 16)
    guard = keep.tile([P, TAILB, 16], BF16, tag="kv_guard")
    nc.sync.dma_start(
        guard[:], kv_r[:, nblk_all - TAILB : nblk_all, 0:16]
    )
    nc.gpsimd.tensor_copy(guard[:, 0, :], guard[:, 1, :])
    lmax = max(g["L"] for g in groups)
    with (
        tc.tile_pool(name="eg_gather", bufs=3) as eg_gather,
        tc.tile_pool(name="eg_tt", bufs=2) as eg_tt,
        tc.tile_pool(name="eg_qps", bufs=2, space="PSUM") as eg_qps,
        tc.tile_pool(name="eg_wave", bufs=3) as eg_wave,
        tc.tile_pool(name="eg_scps", bufs=2, space="PSUM") as eg_scps,
        tc.tile_pool(name="eg_ftps", bufs=c.grp + 1, space="PSUM") as eg_ftps,
    ):
        olo = ohi = 0
        for g in groups:
            L, L_lo, L_hi = g["L"], g["L_lo"], g["L_hi"]
            kT = eg_gather.tile([P, 1, lmax], BF16, tag="kT")
            qT = eg_gather.tile([P, 1, lmax], BF16, tag="qT")
            vE = eg_gather.tile([P, lmax // P, P], BF16, tag="vE")
            Ssb = eg_gather.tile([P, lmax], BF16, tag="Ssb")
            STsb = eg_gather.tile([P, lmax], BF16, tag="STsb")
            nc.sync.dma_start(
                Ssb[:, 0:L], t["S_all"][:, g["scol"] : g["scol"] + L]
            )
            nc.sync.dma_start(
                STsb[:, 0:L], t["ST_all"][:, g["scol"] : g["scol"] + L]
            )
            if L_lo:
                gather_chunked(
                    lambda o, m: kT[:, :, o : o + m],
                    kv_table[:][0 : c.half, 0 : c.feats],
                    kvlo, olo, L_lo, c.feats, 2 * c.feats, True,
                )
                gather_chunked(
                    lambda o, m: vE[:, o // P : (o + m) // P, :],
                    kv_table[:][0 : c.half, c.feats : 2 * c.feats],
                    kvlo, olo, L_lo, c.feats, 2 * c.feats, False,
                )
            if L_hi:
                gather_chunked(
                    lambda o, m: kT[:, :, L_lo + o : L_lo + o + m],
                    kv_table[:][c.half : c.npad, 0 : c.feats],
                    kvhi, ohi, L_hi, c.feats, 2 * c.feats, True,
                )
                gather_chunked(
                    lambda o, m: vE[:, (L_lo + o) // P : (L_lo + o + m) // P, :],
                    kv_table[:][c.half : c.npad, c.feats : 2 * c.feats],
                    kvhi, ohi, L_hi, c.feats, 2 * c.feats, False,
                )
            olo += L_lo
            ohi += L_hi

            # qT per edge via one-hot select matmul (replaces the q gather):
            # qT[f, e] = sum_d q_blk[d, f] * ST[d, e]
            for r in g["runs"]:
                b = r["block"]
                for c0 in range(0, r["ncols"], c.qsel_chunk):
                    w = min(c.qsel_chunk, r["ncols"] - c0)
                    qps = eg_qps.tile([P, c.qsel_chunk], FP32, tag="qps")
                    nc.tensor.matmul(
                        qps[:, 0:w],
                        q_sb[:, b, :],
                        STsb[:, r["col"] + c0 : r["col"] + c0 + w],
                        start=True,
                        stop=True,
                    )
                    nc.any.tensor_copy(
                        qT[:, 0, r["col"] + c0 : r["col"] + c0 + w], qps[:, 0:w]
                    )

            TT = eg_tt.tile([P, lmax], BF16, tag="TT")
            for off in range(0, L, c.tmult_chunk):
                w = min(c.tmult_chunk, L - off)
                nc.vector.tensor_tensor(
                    TT[:, off : off + w],
                    kT[:, 0, off : off + w],
                    qT[:, 0, off : off + w],
                    op=OP.mult,
                )
            if c.debug and g is groups[0]:
                nc.sync.dma_start(t["dbg_kT"][:, 0:L], kT[:, 0, 0:L])
                nc.sync.dma_start(t["dbg_qT"][:, 0:L], qT[:, 0, 0:L])
                nc.sync.dma_start(
                    t["dbg_vE"][:, 0 : L // P, :], vE[:, 0 : L // P, :]
                )

            ftps = {}
            subs = g["subs"]
            for w0 in range(0, len(subs), c.wave):
                wsubs = subs[w0 : w0 + c.wave]
                wl = len(wsubs)
                scps = eg_scps.tile([P, c.wave * c.heads], FP32, tag="scps")
                for i, s in enumerate(wsubs):
                    nc.tensor.matmul(
                        scps[:, i * c.heads : (i + 1) * c.heads],
                        TT[:, s["col"] : s["col"] + P],
                        Rm[:],
                        start=True,
                        stop=True,
                    )
                pexp = eg_wave.tile([P, c.wave, c.heads], BF16, tag="pexp")
                nc.scalar.activation(
                    pexp[:, 0:wl],
                    scps[:, 0 : wl * c.heads].rearrange("p (a h) -> p a h", h=c.heads),
                    AF.Exp,
                    scale=scale,
                )
                Mt = eg_wave.tile([P, c.wave, P], BF16, tag="Mt")
                nc.vector.tensor_tensor(
                    Mt[:, 0:wl].rearrange("p a (h d) -> p a h d", d=c.dhead),
                    vE[:, w0 : w0 + wl, :].rearrange("p a (h d) -> p a h d", d=c.dhead),
                    pexp[:, 0:wl]
                    .rearrange("p a (h o) -> p a h o", o=1)
                    .to_broadcast([P, wl, c.heads, c.dhead]),
                    op=OP.mult,
                )
                for i, s in enumerate(wsubs):
                    b = s["block"]
                    if s["first"]:
                        ftps[b] = eg_ftps.tile(
                            [P, 136], FP32, tag="ftps", name="ftps"
                        )
                    nc.tensor.matmul(
                        ftps[b][:, 0:128],
                        Ssb[:, s["col"] : s["col"] + P],
                        Mt[:, i, :],
                        start=s["first"],
                        stop=s["last"],
                        skip_group_check=True,
                    )
                    # ft2's start already marked this bank pending-zero, so
                    # the first denom write lands on zeroed bytes with start=False
                    nc.tensor.matmul(
                        ftps[b][:, 128:136],
                        Ssb[:, s["col"] : s["col"] + P],
                        pexp[:, i, :],
                        start=False,
                        stop=s["last"],
                        skip_group_check=True,
                    )
                    if s["last"]:
                        nc.any.tensor_copy(ftden_sb[:, b, :], ftps[b][:])

    # ---------- phase 3: epilogue (divide, residual, LN1, FFN, LN2) ----------
    def layernorm(pool, x32, nb, out_dtype=FP32):
        """x32: [P, nb, 128] fp32 SBUF tile -> normalized * g + b (new tile)."""
        msum = pool.tile([P, c.grp], FP32, tag="ln_msum")
        nc.vector.tensor_reduce(
            msum[:, 0:nb], x32[:, 0:nb, :], axis=mybir.AxisListType.X, op=OP.add
        )
        nmean = pool.tile([P, c.grp], FP32, tag="ln_nmean")
        nc.vector.tensor_scalar_mul(nmean[:, 0:nb], msum[:, 0:nb], -1.0 / c.feats)
        sq = pool.tile([P, c.grp, P], FP32, tag="ln_sq")
        for b in range(nb):
            nc.scalar.activation(
                sq[:, b],
                x32[:, b],
                AF.Square,
                bias=nmean[:, b : b + 1],
            )
        var = pool.tile([P, c.grp], FP32, tag="ln_var")
        nc.vector.tensor_reduce(
            var[:, 0:nb], sq[:, 0:nb, :], axis=mybir.AxisListType.X, op=OP.add
        )
        rstd = pool.tile([P, c.grp], FP32, tag="ln_rstd")
        nc.vector.tensor_scalar(
            rstd[:, 0:nb], var[:, 0:nb], 1.0 / c.feats, 1e-5, op0=OP.mult, op1=OP.add
        )
        nc.vector.reciprocal(rstd[:, 0:nb], rstd[:, 0:nb])
        nc.scalar.sqrt(rstd[:, 0:nb], rstd[:, 0:nb])
        nmr = pool.tile([P, c.grp], FP32, tag="ln_nmr")
        nc.vector.tensor_tensor(
            nmr[:, 0:nb], nmean[:, 0:nb], rstd[:, 0:nb], op=OP.mult
        )
        normed = pool.tile([P, c.grp, P], FP32, tag="ln_normed")
        for b in range(nb):
            nc.scalar.activation(
                normed[:, b],
                x32[:, b],
                AF.Identity,
                scale=rstd[:, b : b + 1],
                bias=nmr[:, b : b + 1],
            )
        out = pool.tile([P, c.grp, P], out_dtype, tag="ln_out" + str(out_dtype))
        nc.vector.tensor_tensor(
            out[:, 0:nb],
            normed[:, 0:nb],
            grep[:].rearrange("p (o f) -> p o f", o=1).to_broadcast([P, nb, P]),
            op=OP.mult,
        )
        nc.vector.tensor_tensor(
            out[:, 0:nb],
            out[:, 0:nb],
            brep[:].rearrange("p (o f) -> p o f", o=1).to_broadcast([P, nb, P]),
            op=OP.add,
        )
        return out

    if c.debug:
        nc.sync.dma_start(t["dbg_ftden"][:], ftden_sb[:])
    with (
        tc.tile_pool(name="ep", bufs=2) as ep,
        tc.tile_pool(name="ep_ps", bufs=2, space="PSUM") as ep_ps,
        tc.tile_pool(name="ep_h1ps", bufs=2, space="PSUM") as ep_h1ps,
    ):
        for g in groups:
            bs = g["bs"]
            nb = len(bs)
            b0 = bs[0]
            f32 = ep.tile([P, c.grp, P], FP32, tag="f32")
            nc.sync.dma_start(
                f32[:, 0:nb, :],
                t["feat32_loc"][:]
                .rearrange("(s p) f -> p s f", p=P)[:, b0 : b0 + nb, :],
            )
            r = ep.tile([P, c.grp, c.heads], FP32, tag="recip")
            nc.vector.tensor_scalar_max(
                r[:, 0:nb], ftden_sb[:, b0 : b0 + nb, 128:136], 1e-30
            )
            nc.vector.reciprocal(r[:, 0:nb], r[:, 0:nb])
            rst = ep.tile([P, c.grp, P], FP32, tag="rst")
            nc.vector.tensor_tensor(
                rst[:, 0:nb],
                ftden_sb[:, b0 : b0 + nb, 0:128].rearrange(
                    "p s (h d) -> p s h d", d=c.dhead
                ),
                r[:, 0:nb].rearrange("p s (h o) -> p s h o", o=1).to_broadcast(
                    [P, nb, c.heads, c.dhead]
                ),
                op=OP.mult,
            )
            nc.vector.tensor_tensor(
                rst[:, 0:nb], rst[:, 0:nb], f32[:, 0:nb, :], op=OP.add
            )
            ln1 = layernorm(ep, rst, nb)
            ln1b = ep.tile([P, c.grp, P], BF16, tag="ln1b")
            nc.scalar.copy(ln1b[:, 0:nb], ln1[:, 0:nb])
            # transpose ln1 -> feat-major for FFN
            rT_ps = ep_ps.tile([P, c.grp * P], BF16, tag="rT_ps")
            for b in range(nb):
                nc.tensor.transpose(
                    rT_ps[:, b * P : (b + 1) * P], ln1b[:, b, :], ident[:]
                )
            rT = ep.tile([P, c.grp * P], BF16, tag="rT")
            nc.vector.tensor_copy(rT[:, 0 : nb * P], rT_ps[:, 0 : nb * P])
            # H1 = W1.T @ rT  (feat-major, nh slices) ; prelu via W2a/W2b trick
            ffps = ep_ps.tile([P, c.grp * P], FP32, tag="ffps")
            for h in range(nh):
                h1ps = ep_h1ps.tile([P, c.grp * P], FP32, tag="h1ps")
                nc.tensor.matmul(
                    h1ps[:, 0 : nb * P],
                    w1[:, h * P : (h + 1) * P],
                    rT[:, 0 : nb * P],
                    start=True,
                    stop=True,
                )
                h1sb = ep.tile([P, c.grp * P], BF16, tag="h1sb")
                nc.scalar.activation(
                    h1sb[:, 0 : nb * P],
                    h1ps[:, 0 : nb * P],
                    AF.Identity,
                    bias=b1t[:, h : h + 1],
                )
                habs = ep.tile([P, c.grp * P], BF16, tag="habs")
                nc.scalar.activation(
                    habs[:, 0 : nb * P],
                    h1ps[:, 0 : nb * P],
                    AF.Abs,
                    bias=b1t[:, h : h + 1],
                )
                for b in range(nb):
                    nc.tensor.matmul(
                        ffps[:, b * P : (b + 1) * P],
                        h1sb[:, b * P : (b + 1) * P],
                        w2a[:, h, :],
                        start=(h == 0 and b == 0),
                        stop=False,
                        skip_group_check=True,
                    )
                    nc.tensor.matmul(
                        ffps[:, b * P : (b + 1) * P],
                        habs[:, b * P : (b + 1) * P],
                        w2b[:, h, :],
                        start=False,
                        stop=(h == nh - 1),
                        skip_group_check=True,
                    )
            rst2 = ep.tile([P, c.grp, P], FP32, tag="rst2")
            nc.vector.tensor_tensor(
                rst2[:, 0:nb],
                ffps[:, 0 : nb * P].rearrange("p (s f) -> p s f", f=P),
                ln1[:, 0:nb],
                op=OP.add,
            )
            nc.vector.tensor_tensor(
                rst2[:, 0:nb],
                rst2[:, 0:nb],
                b2rep[:].rearrange("p (o f) -> p o f", o=1).to_broadcast([P, nb, P]),
                op=OP.add,
            )
            ln2 = layernorm(ep, rst2, nb)
            nc.sync.dma_start(
                t["out"][:].rearrange("(s p) f -> p s f", p=P)[:, b0 : b0 + nb, :],
                ln2[:, 0:nb],
            )


def _build(meta, cfg: GATCfg):
    c = cfg
    nc = bacc.Bacc("TRN2", target_bir_lowering=False, debug=False, num_devices=c.n_cores)
    t = {}

    def inp(name, shape, dtype):
        t[name] = nc.dram_tensor(name, shape, dtype, kind="ExternalInput").ap()

    inp("feat16", [c.npad, c.feats], BF16)
    inp("feat16_loc", [c.local_pad, c.feats], BF16)
    inp("feat32_loc", [c.local_pad, c.feats], FP32)
    inp("wq", [c.feats, c.feats], BF16)
    inp("wkv", [c.feats, 2 * c.feats], BF16)
    inp("w1", [c.feats, c.dff], BF16)
    inp("w2a", [P, c.dff // P, c.feats], BF16)
    inp("w2b", [P, c.dff // P, c.feats], BF16)
    inp("b1t", [P, c.dff // P], FP32)
    inp("b2rep", [P, c.feats], FP32)
    inp("grep", [P, c.feats], FP32)
    inp("brep", [P, c.feats], FP32)
    inp("Rm", [c.feats, c.heads], BF16)
    inp("ident", [P, P], BF16)
    inp("kv_idx_lo", [P, max(meta["tot_lo"] // 16, 1)], I16)
    inp("kv_idx_hi", [P, max(meta["tot_hi"] // 16, 1)], I16)
    inp("S_all", [P, meta["tot_cols"]], BF16)
    inp("ST_all", [P, meta["tot_cols"]], BF16)
    t["out"] = nc.dram_tensor(
        "out", [c.local_pad, c.feats], FP32, kind="ExternalOutput"
    ).ap()
    if c.debug:
        t["dbg_ftden"] = nc.dram_tensor(
            "dbg_ftden", [P, c.nblk, 136], FP32, kind="ExternalOutput"
        ).ap()
        lmax = max(g["L"] for g in meta["groups"])
        t["dbg_kT"] = nc.dram_tensor(
            "dbg_kT", [P, lmax], BF16, kind="ExternalOutput"
        ).ap()
        t["dbg_qT"] = nc.dram_tensor(
            "dbg_qT", [P, lmax], BF16, kind="ExternalOutput"
        ).ap()
        t["dbg_vE"] = nc.dram_tensor(
            "dbg_vE", [P, lmax // P, P], BF16, kind="ExternalOutput"
        ).ap()

    with tile.TileContext(nc) as tc:
        _emit(tc, t, meta, cfg)
    nc.compile()
    return nc


def _in_maps(meta, streams, shared, cfg: GATCfg):
    maps = []
    for ci in range(cfg.n_cores):
        m = dict(shared)
        st = streams[ci]
        feat32_loc = st["feat32_loc"]
        m["feat16_loc"] = feat32_loc.astype(bf16)
        m["feat32_loc"] = feat32_loc
        m["kv_idx_lo"] = (
            st["kv_idx_lo"]
            if meta["tot_lo"]
            else np.zeros((P, 1), np.int16)
        )
        m["kv_idx_hi"] = (
            st["kv_idx_hi"]
            if meta["tot_hi"]
            else np.zeros((P, 1), np.int16)
        )
        m["S_all"] = st["S_all"]
        m["ST_all"] = st["ST_all"]
        maps.append(m)
    return maps


_CACHE = {}


def kernel(**inputs) -> np.ndarray:
    cfg = GATCfg()
    meta, streams, shared = _prep(inputs, cfg)
    key = "real"
    if key not in _CACHE:
        _CACHE[key] = _build(meta, cfg)
    nc = _CACHE[key]
    maps = _in_maps(meta, streams, shared, cfg)
    res = run_bass_kernel_spmd(nc, maps, core_ids=list(range(cfg.n_cores)))
    out = np.empty((cfg.n_nodes, cfg.feats), np.float32)
    for ci in range(cfg.n_cores):
        out[ci * cfg.npc : (ci + 1) * cfg.npc] = res.results[ci]["out"][: cfg.npc]
    return out


# revision 25
# speedup vs baseline: 3.6955x; 1.8003x over previous
"""GAT message-passing layer on 8 Trainium2 NeuronCores (Bass/Tile).

Strategy (matches the sharding hint): nodes are partitioned across the 8
cores; each edge is owned by the core that owns its destination node, so the
segment softmax and the weighted scatter-sum stay core-local.  Every core
computes the bf16 k/v projection table for all nodes (cheap, replicated;
natural-layout loads + PE transposes — no slow DMA-transpose) and keeps q for
its local nodes in SBUF.  Per-edge k rows are fetched feat-major with SWDGE
`dma_gather` (transpose mode), v rows edge-major (row mode); per-edge q is NOT
gathered — within a sub all 128 edges share one dst block, so qT per edge is a
one-hot select matmul q_blk^T @ ST against the streamed ST matrix.  Scores are
reduced on the PE with a block-diagonal head selector, the segment softmax
numerator/denominator are accumulated in PSUM via one-hot scatter matmuls, and
the epilogue (divide, residual, LN, FFN with PReLU folded into two weight
matrices, LN) runs per 128-node block.
"""

import sys

sys.path.insert(0, "/opt/trn_rl_repo")

import math
import os
from contextlib import ExitStack
from dataclasses import dataclass, field

import numpy as np
import ml_dtypes

import concourse.bass as bass
import concourse.bacc as bacc
import concourse.mybir as mybir
import concourse.tile as tile
from concourse._compat import with_exitstack
from concourse.bass_utils import run_bass_kernel_spmd
from concourse.library_config import mlp as mlp_lib

bf16 = ml_dtypes.bfloat16
P = 128
AF = mybir.ActivationFunctionType
OP = mybir.AluOpType
FP32 = mybir.dt.float32
BF16 = mybir.dt.bfloat16
I16 = mybir.dt.int16


@dataclass
class GATCfg:
    n_nodes: int = 50000
    n_edges: int = 640000
    feats: int = 128
    heads: int = 8
    dhead: int = 16
    dff: int = 512
    n_cores: int = 8
    grp: int = 2  # dst blocks per gather group
    wave: int = 4  # 128-edge subchunks per score/message wave
    tmult_chunk: int = 2048
    qsel_chunk: int = 512  # qT select matmul column chunk (1 PSUM bank)
    debug: bool = False

    @property
    def npc(self):  # nodes per core
        return self.n_nodes // self.n_cores

    @property
    def nblk(self):  # local 128-node blocks per core
        return (self.npc + P - 1) // P

    @property
    def local_pad(self):
        return self.nblk * P

    @property
    def npad(self):  # padded global node count (k/v table rows)
        return ((self.n_nodes + P - 1) // P) * P

    @property
    def half(self):  # int16 index split point (row offset base)
        h = self.npad // 2
        assert h < 32768 and (self.npad - h) <= 32768
        return h

    @property
    def ngrp(self):
        return (self.nblk + self.grp - 1) // self.grp


def _wrap16(idx):
    """int16 index list -> [128, n/16] SWDGE layout (16-wrap, replicated x8)."""
    idx = np.asarray(idx, np.int16)
    n = len(idx)
    assert n % 16 == 0
    return np.tile(idx.reshape(n // 16, 16).T, (8, 1)).copy()


def _prep(inputs, cfg: GATCfg):
    """Host-side graph partitioning / padding / index+S-matrix construction."""
    c = cfg
    feat = np.asarray(inputs["feat"], np.float32)
    src = np.asarray(inputs["src"], np.int64)
    dst = np.asarray(inputs["dst"], np.int64)

    feat_pad = np.zeros((c.npad, c.feats), np.float32)
    feat_pad[: c.n_nodes] = feat
    feat16 = feat_pad.astype(bf16)

    # ---- per (core, block, half) edge lists ----
    core_of = dst // c.npc
    per_core = []
    for ci in range(c.n_cores):
        sel = np.nonzero(core_of == ci)[0]
        dloc = dst[sel] - ci * c.npc
        blk = dloc // P
        half = (src[sel] >= c.half).astype(np.int64)
        order = np.lexsort((dloc, half, blk))
        sel, dloc, blk, half = sel[order], dloc[order], blk[order], half[order]
        lists = {}
        for b in range(c.nblk):
            for h in range(2):
                m = (blk == b) & (half == h)
                lists[(b, h)] = (src[sel[m]], dloc[m])
        per_core.append(lists)

    # uniform sub-chunk counts across cores
    n_sub = np.zeros((c.nblk, 2), np.int64)
    for b in range(c.nblk):
        for h in range(2):
            mx = max(len(per_core[ci][(b, h)][0]) for ci in range(c.n_cores))
            n_sub[b, h] = (mx + P - 1) // P

    # ---- group structure (shared across cores) ----
    groups = []  # list of dicts with static metadata
    scol = 0
    for g in range(c.ngrp):
        bs = list(range(g * c.grp, min((g + 1) * c.grp, c.nblk)))
        L_lo = int(sum(n_sub[b, 0] for b in bs)) * P
        L_hi = int(sum(n_sub[b, 1] for b in bs)) * P
        subs = []
        runs = []  # contiguous (block, col, ncols) spans for the qT select
        # per-block first/last sub bookkeeping (block's subs = its lo + hi subs)
        tot_per_block = {b: int(n_sub[b, 0] + n_sub[b, 1]) for b in bs}
        seen = {b: 0 for b in bs}
        s_idx = 0
        for h in range(2):
            for b in bs:
                ns = int(n_sub[b, h])
                if ns:
                    runs.append(dict(block=b, col=s_idx * P, ncols=ns * P))
                for _ in range(ns):
                    seen[b] += 1
                    subs.append(
                        dict(
                            block=b,
                            col=s_idx * P,
                            first=seen[b] == 1,
                            last=seen[b] == tot_per_block[b],
                        )
                    )
                    s_idx += 1
        groups.append(
            dict(
                bs=bs, L_lo=L_lo, L_hi=L_hi, L=L_lo + L_hi, subs=subs,
                runs=runs, scol=scol,
            )
        )
        scol += L_lo + L_hi

    tot_cols = scol
    tot_lo = sum(g["L_lo"] for g in groups)
    tot_hi = sum(g["L_hi"] for g in groups)

    meta = dict(groups=groups, tot_cols=tot_cols, tot_lo=tot_lo, tot_hi=tot_hi)

    # ---- per-core streams ----
    per_core_streams = []
    for ci in range(c.n_cores):
        kv_lo = np.zeros(tot_lo, np.int16)
        kv_hi = np.zeros(tot_hi, np.int16)
        S = np.zeros((P, tot_cols), np.float32)
        ST = np.zeros((P, tot_cols), np.float32)
        olo = ohi = 0
        for g in groups:
            gcol = g["scol"]
            i = 0  # edge position within group tile
            for h in range(2):
                for b in g["bs"]:
                    s_arr, d_arr = per_core[ci][(b, h)]
                    npadded = int(n_sub[b, h]) * P
                    rel = np.zeros(npadded, np.int16)
                    rel[: len(s_arr)] = (s_arr - (c.half if h else 0)).astype(
                        np.int16
                    )
                    if h == 0:
                        kv_lo[olo : olo + npadded] = rel
                        olo += npadded
                    else:
                        kv_hi[ohi : ohi + npadded] = rel
                        ohi += npadded
                    # one-hot S: edge j (pos i+j) -> col 128*s + (dloc - b*128)
                    jj = np.arange(len(d_arr))
                    pos = i + jj
                    ss = pos // P
                    pp = pos % P
                    S[pp, gcol + ss * P + (d_arr - b * P)] = 1.0
                    # one-hot ST (transposed layout): row = dst slot, col = pos
                    ST[d_arr - b * P, gcol + pos] = 1.0
                    i += npadded
        feat32_loc = np.zeros((c.local_pad, c.feats), np.float32)
        feat32_loc[: c.npc] = feat[ci * c.npc : (ci + 1) * c.npc]
        per_core_streams.append(
            dict(
                kv_idx_lo=_wrap16(kv_lo),
                kv_idx_hi=_wrap16(kv_hi),
                S_all=S.astype(bf16),
                ST_all=ST.astype(bf16),
                feat32_loc=feat32_loc,
            )
        )

    # ---- shared weight/constant tensors ----
    W1 = np.asarray(inputs["W1"], np.float32)
    W2 = np.asarray(inputs["W2"], np.float32)
    a = np.asarray(inputs["prelu_a"], np.float32)
    # prelu(x) = max(x,0) + a*min(x,0) = ((1+a)/2)*x + ((1-a)/2)*|x|
    nh = c.dff // P
    # [dff, F] -> [P, nh, F] so each head-slice is an SBUF [128 x F] lhsT
    W2a = (
        (((1.0 + a) / 2.0)[:, None] * W2)
        .reshape(nh, P, c.feats)
        .transpose(1, 0, 2)
        .astype(bf16)
    )
    W2b = (
        (((1.0 - a) / 2.0)[:, None] * W2)
        .reshape(nh, P, c.feats)
        .transpose(1, 0, 2)
        .astype(bf16)
    )
    wkv = np.concatenate(
        [np.asarray(inputs["Wk"], np.float32), np.asarray(inputs["Wv"], np.float32)],
        axis=1,
    )
    shared = dict(
        feat16=feat16,
        wq=np.asarray(inputs["Wq"], np.float32).astype(bf16),
        wkv=wkv.astype(bf16),
        w1=W1.astype(bf16),
        w2a=W2a,
        w2b=W2b,
        b1t=np.ascontiguousarray(
            np.asarray(inputs["b1"], np.float32).reshape(nh, P).T
        ),
        b2rep=np.tile(np.asarray(inputs["b2"], np.float32)[None, :], (P, 1)),
        grep=np.tile(np.asarray(inputs["ln1_g"], np.float32)[None, :], (P, 1)),
        brep=np.tile(np.asarray(inputs["ln1_b"], np.float32)[None, :], (P, 1)),
        ident=np.eye(P, dtype=np.float32).astype(bf16),
    )
    return meta, per_core_streams, shared


@with_exitstack
def _emit(ctx: ExitStack, tc: tile.TileContext, t, meta, cfg: GATCfg):
    """Emit the per-core program. `t` maps tensor name -> DRAM AP."""
    c = cfg
    nc = tc.nc
    groups = meta["groups"]
    nh = c.dff // P
    scale = 1.0 / math.sqrt(c.heads * c.dhead)

    with tc.tile_critical():
        nc.gpsimd.load_library(mlp_lib)

    # ---------- persistent pool: constants, indices, q + ft2 storage ----------
    keep = ctx.enter_context(tc.tile_pool(name="keep", bufs=1))

    def load_const(name, shape, dtype):
        tl = keep.tile(shape, dtype, tag=name)
        nc.sync.dma_start(tl[:], t[name][:])
        return tl

    wq = load_const("wq", [P, P], BF16)
    wkv = load_const("wkv", [P, 2 * c.feats], BF16)
    w1 = load_const("w1", [P, c.dff], BF16)
    w2a = load_const("w2a", [P, nh, c.feats], BF16)
    w2b = load_const("w2b", [P, nh, c.feats], BF16)
    b1t = load_const("b1t", [P, nh], FP32)
    b2rep = load_const("b2rep", [P, P], FP32)
    grep = load_const("grep", [P, P], FP32)
    brep = load_const("brep", [P, P], FP32)
    ident = load_const("ident", [P, P], BF16)
    kvlo = load_const("kv_idx_lo", [P, max(meta["tot_lo"] // 16, 1)], I16)
    kvhi = load_const("kv_idx_hi", [P, max(meta["tot_hi"] // 16, 1)], I16)

    q_sb = keep.tile([P, c.nblk, c.feats], BF16, tag="q_sb")
    ftden_sb = keep.tile([P, c.nblk, 136], FP32, tag="ftden_sb")

    dram = ctx.enter_context(tc.tile_pool(name="dram", bufs=1, space="DRAM"))
    kv_table = dram.tile([c.npad, 2 * c.feats], BF16)

    # ---------- phase 1: projection tables ----------
    # natural-layout loads + PE transpose (DMA-transpose is ~4us/tile serialized)
    feat_r = t["feat16"][:].rearrange("(s p) f -> p s f", p=P)
    kv_r = kv_table[:].rearrange("(s p) f -> p s f", p=P)
    PB = 2  # blocks per DMA batch
    with (
        tc.tile_pool(name="prj_ft", bufs=3) as prj_ft,
        tc.tile_pool(name="prj_tps", bufs=3, space="PSUM") as prj_tps,
        tc.tile_pool(name="prj_tsb", bufs=4) as prj_tsb,
        tc.tile_pool(name="prj_ps", bufs=3, space="PSUM") as prj_ps,
        tc.tile_pool(name="prj_sb", bufs=3) as prj_sb,
    ):
        nbl = c.npad // P
        n_kv_stores = (nbl + PB - 1) // PB
        for pair in range(n_kv_stores):
            blks = range(pair * PB, min((pair + 1) * PB, nbl))
            nb_ = len(blks)
            ft = prj_ft.tile([P, PB, P], BF16, tag="ft")
            nc.sync.dma_start(
                ft[:, 0:nb_, :], feat_r[:, pair * PB : pair * PB + nb_, :]
            )
            sb = prj_sb.tile([P, PB, 2 * c.feats], BF16, tag="kvsb")
            for s in range(nb_):
                tps = prj_tps.tile([P, P], BF16, tag="tps")
                nc.tensor.transpose(tps[:], ft[:, s, :], ident[:])
                tsb = prj_tsb.tile([P, P], BF16, tag="tsb")
                nc.any.tensor_copy(tsb[:], tps[:])
                ps = prj_ps.tile([P, 2 * c.feats], FP32, tag="kvps")
                nc.tensor.matmul(ps[:], tsb[:], wkv[:], start=True, stop=True)
                nc.any.tensor_copy(sb[:, s, :], ps[:])
            nc.sync.dma_start(
                kv_r[:, pair * PB : pair * PB + nb_, :], sb[:, 0:nb_, :]
            )
        # local q -> SBUF only (no DRAM round-trip)
        floc_r = t["feat16_loc"][:].rearrange("(s p) f -> p s f", p=P)
        for pair in range((c.nblk + PB - 1) // PB):
            blks = range(pair * PB, min((pair + 1) * PB, c.nblk))
            ft = prj_ft.tile([P, PB, P], BF16, tag="ft")
            nc.sync.dma_start(
                ft[:, 0 : len(blks), :], floc_r[:, pair * PB : pair * PB + len(blks), :]
            )
            for s, blk in enumerate(blks):
                tps = prj_tps.tile([P, P], BF16, tag="tps")
                nc.tensor.transpose(tps[:], ft[:, s, :], ident[:])
                tsb = prj_tsb.tile([P, P], BF16, tag="tsb")
                nc.any.tensor_copy(tsb[:], tps[:])
                ps = prj_ps.tile([P, 2 * c.feats], FP32, tag="kvps")
                nc.tensor.matmul(
                    ps[:, 0 : c.feats], tsb[:], wq[:], start=True, stop=True
                )
                nc.any.tensor_copy(q_sb[:, blk, :], ps[:, 0 : c.feats])

    def gather_rows(out_ap, in_ap, idx_tile, idx_off, n):
        nc.gpsimd.dma_gather(
            out_ap,
            in_ap,
            idx_tile[:, idx_off // 16 : (idx_off + n) // 16],
            n,
            n,
            2 * c.feats,
            elem_step=2 * c.feats,
            transpose=False,
            single_packet=False,
        )

    # ---------- phase 2: two-sweep edge processing ----------
    smax_h = max(max(g["L_lo"], g["L_hi"]) for g in groups) // P
    EPB = c.grp  # blocks per epilogue call (one gather group)

    with (
        tc.tile_pool(name="eg_kv", bufs=3) as eg_kv,
        tc.tile_pool(name="eg_s", bufs=2) as eg_s,
        tc.tile_pool(name="eg_tt", bufs=2) as eg_tt,
        tc.tile_pool(name="ep", bufs=2) as ep,
        tc.tile_pool(name="eg_qps", bufs=2, space="PSUM") as eg_qps,
        tc.tile_pool(name="eg_ftps", bufs=2, space="PSUM") as eg_ftps,
    ):

        def sweep_group(g, h, off):
            """Process one group's lo (h=0) or hi (h=1) edges; returns new off."""
            Lh = g["L_lo"] if h == 0 else g["L_hi"]
            colrel = 0 if h == 0 else g["L_lo"]
            base = g["scol"] + colrel
            ns = Lh // P
            kvE = eg_kv.tile([P, smax_h, 2 * c.feats], BF16, tag="kvE")
            Ssb = eg_s.tile([P, smax_h * P], BF16, tag="Ssb")
            STsb = eg_s.tile([P, smax_h * P], BF16, tag="STsb")
            nc.sync.dma_start(Ssb[:, 0:Lh], t["S_all"][:, base : base + Lh])
            nc.sync.dma_start(STsb[:, 0:Lh], t["ST_all"][:, base : base + Lh])
            gather_rows(
                kvE[:, 0:ns, :],
                kv_table[:][0 : c.half, :]
                if h == 0
                else kv_table[:][c.half : c.npad, :],
                kvlo if h == 0 else kvhi,
                off,
                Lh,
            )
            runs_h = [r for r in g["runs"] if r["half"] == h]
            # per-sub: qE select (edge-major) + per-edge k*q products
            # qE[e, f] = sum_d ST[d, e] * q_blk[d, f]
            # qE selects batched 4 subs per PSUM bank so one Vector mult
            # covers 4 subs (per-sub mults were overhead-dominated)
            TT = eg_tt.tile([P, smax_h, P], BF16, tag="TT")
            QB = 4
            for r in runs_h:
                c0 = r["col"] - colrel
                nsr = r["ncols"] // P
                for k0 in range(0, nsr, QB):
                    w = min(QB, nsr - k0)
                    qps = eg_qps.tile([P, QB, P], FP32, tag="qps")
                    for k in range(k0, k0 + w):
                        nc.tensor.matmul(
                            qps[:, k - k0, :],
                            STsb[:, c0 + k * P : c0 + (k + 1) * P],
                            q_sb[:, r["block"], :],
                            start=True,
                            stop=True,
                        )
                    si0 = c0 // P + k0
                    nc.vector.tensor_tensor(
                        TT[:, si0 : si0 + w, :],
                        kvE[:, si0 : si0 + w, 0 : c.feats],
                        qps[:, 0:w, :],
                        op=OP.mult,
                    )
            # group-half-wide: per-head score reduce, exp, weighted messages
            scores = eg_tt.tile([P, smax_h * c.heads], FP32, tag="scores")
            nc.vector.tensor_reduce(
                scores[:, 0 : ns * c.heads],
                TT[:, 0:ns, :].rearrange("p a (h d) -> p (a h) d", d=c.dhead),
                axis=mybir.AxisListType.X,
                op=OP.add,
            )
            pexp = eg_tt.tile([P, smax_h * c.heads], BF16, tag="pexp")
            nc.scalar.activation(
                pexp[:, 0 : ns * c.heads],
                scores[:, 0 : ns * c.heads],
                AF.Exp,
                scale=scale,
            )
            Mt = eg_tt.tile([P, smax_h, P], BF16, tag="Mt")
            nc.vector.tensor_tensor(
                Mt[:, 0:ns].rearrange("p a (h d) -> p a h d", d=c.dhead),
                kvE[:, 0:ns, c.feats : 2 * c.feats].rearrange(
                    "p a (h d) -> p a h d", d=c.dhead
                ),
                pexp[:, 0 : ns * c.heads]
                .rearrange("p (a h o) -> p a h o", h=c.heads, o=1)
                .to_broadcast([P, ns, c.heads, c.dhead]),
                op=OP.mult,
            )
            for r in runs_h:
                b = r["block"]
                c0 = r["col"] - colrel
                nsr = r["ncols"] // P
                ftp = eg_ftps.tile([P, 136], FP32, tag="ftps", name="ftps")
                for k in range(nsr):
                    si = c0 // P + k
                    nc.tensor.matmul(
                        ftp[:, 0:128],
                        Ssb[:, c0 + k * P : c0 + (k + 1) * P],
                        Mt[:, si, :],
                        start=k == 0,
                        stop=k == nsr - 1,
                        skip_group_check=True,
                    )
                    # ft2's start already marked this bank pending-zero, so
                    # the first denom write lands on zeroed bytes (start=False)
                    nc.tensor.matmul(
                        ftp[:, 128:136],
                        Ssb[:, c0 + k * P : c0 + (k + 1) * P],
                        pexp[:, si * c.heads : (si + 1) * c.heads],
                        start=False,
                        stop=k == nsr - 1,
                        skip_group_check=True,
                    )
                nc.scalar.copy(
                    (ftden_lo if h == 0 else ftden_sb)[:, b, :], ftp[:]
                )
            if h == 1:
                epilogue(g["bs"][0], len(g["bs"]))
            return off + Lh

        # sweep A: build lo table, then lo gathers with the hi table build
        # interleaved (different engines/rows -> full overlap)
        with (
            tc.tile_pool(name="prj_ft", bufs=3) as prj_ft,
            tc.tile_pool(name="prj_tps", bufs=2, space="PSUM") as prj_tps,
            tc.tile_pool(name="prj_tsb", bufs=2) as prj_tsb,
            tc.tile_pool(name="prj_ps", bufs=2, space="PSUM") as prj_ps,
            tc.tile_pool(name="prj_sb", bufs=3) as prj_sb,
        ):
            prj = (prj_ft, prj_tps, prj_tsb, prj_ps, prj_sb)
            for pair in range(n_pairsA):
                emit_pair(pair, prj)
            for pair in range((c.nblk + PB - 1) // PB):
                emit_q_pair(pair, prj)
            guard_read(lo_blocks - 16, lo_blocks, "guard_lo")
            per = (len(pairsB) + len(groups) - 1) // len(groups)
            pbi = 0
            olo = 0
            for g in groups:
                for _ in range(per):
                    if pbi < len(pairsB):
                        emit_pair(pairsB[pbi], prj)
                        pbi += 1
                olo = sweep_group(g, 0, olo)
            while pbi < len(pairsB):
                emit_pair(pairsB[pbi], prj)
                pbi += 1
        # sweep B: hi gathers + scatter + interleaved epilogue
        guard_read(nbl - 16, nbl, "guard_hi")
        with (
            tc.tile_pool(name="ep_ps", bufs=1, space="PSUM") as ep_ps,
            tc.tile_pool(name="ep_h1ps", bufs=2, space="PSUM") as ep_h1ps,
        ):
            ohi = 0
            for g in groups:
                ohi = sweep_group(g, 1, ohi)

    if c.debug:
        nc.sync.dma_start(t["dbg_ftden"][:], ftden_sb[:])
    with (
        tc.tile_pool(name="ep", bufs=2) as ep,
        tc.tile_pool(name="ep_ps", bufs=2, space="PSUM") as ep_ps,
        tc.tile_pool(name="ep_h1ps", bufs=2, space="PSUM") as ep_h1ps,
    ):
        for b0 in range(0, c.nblk, EPB):
            nb = min(EPB, c.nblk - b0)
            f32 = ep.tile([P, EPB, P], FP32, tag="f32")
            nc.sync.dma_start(
                f32[:, 0:nb, :],
                t["feat32_loc"][:]
                .rearrange("(s p) f -> p s f", p=P)[:, b0 : b0 + nb, :],
            )
            r = ep.tile([P, EPB, c.heads], FP32, tag="recip")
            nc.vector.tensor_scalar_max(
                r[:, 0:nb], ftden_sb[:, b0 : b0 + nb, 128:136], 1e-30
            )
            nc.vector.reciprocal(r[:, 0:nb], r[:, 0:nb])
            rst = ep.tile([P, EPB, P], FP32, tag="rst")
            nc.vector.tensor_tensor(
                rst[:, 0:nb],
                ftden_sb[:, b0 : b0 + nb, 0:128].rearrange(
                    "p s (h d) -> p s h d", d=c.dhead
                ),
                r[:, 0:nb].rearrange("p s (h o) -> p s h o", o=1).to_broadcast(
                    [P, nb, c.heads, c.dhead]
                ),
                op=OP.mult,
            )
            nc.vector.tensor_tensor(
                rst[:, 0:nb], rst[:, 0:nb], f32[:, 0:nb, :], op=OP.add
            )
            ln1 = layernorm(ep, rst, nb)
            ln1b = ep.tile([P, EPB, P], BF16, tag="ln1b")
            nc.scalar.copy(ln1b[:, 0:nb], ln1[:, 0:nb])
            # transpose ln1 -> feat-major for FFN
            rT_ps = ep_ps.tile([P, EPB * P], BF16, tag="rT_ps")
            for b in range(nb):
                nc.tensor.transpose(
                    rT_ps[:, b * P : (b + 1) * P], ln1b[:, b, :], ident[:]
                )
            rT = ep.tile([P, EPB * P], BF16, tag="rT")
            nc.vector.tensor_copy(rT[:, 0 : nb * P], rT_ps[:, 0 : nb * P])
            # H1 = W1.T @ rT  (feat-major, nh slices) ; prelu via W2a/W2b trick
            ffps = ep_ps.tile([P, EPB * P], FP32, tag="ffps")
            for h in range(nh):
                h1ps = ep_h1ps.tile([P, EPB * P], FP32, tag="h1ps")
                nc.tensor.matmul(
                    h1ps[:, 0 : nb * P],
                    w1[:, h * P : (h + 1) * P],
                    rT[:, 0 : nb * P],
                    start=True,
                    stop=True,
                )
                h1sb = ep.tile([P, EPB * P], BF16, tag="h1sb")
                nc.scalar.activation(
                    h1sb[:, 0 : nb * P],
                    h1ps[:, 0 : nb * P],
                    AF.Identity,
                    bias=b1t[:, h : h + 1],
                )
                habs = ep.tile([P, EPB * P], BF16, tag="habs")
                nc.scalar.activation(
                    habs[:, 0 : nb * P],
                    h1ps[:, 0 : nb * P],
                    AF.Abs,
                    bias=b1t[:, h : h + 1],
                )
                for b in range(nb):
                    nc.tensor.matmul(
                        ffps[:, b * P : (b + 1) * P],
                        h1sb[:, b * P : (b + 1) * P],
                        w2a[:, h, :],
                        start=(h == 0 and b == 0),
                        stop=False,
                        skip_group_check=True,
                    )
                    nc.tensor.matmul(
                        ffps[:, b * P : (b + 1) * P],
                        habs[:, b * P : (b + 1) * P],
                        w2b[:, h, :],
                        start=False,
                        stop=(h == nh - 1),
                        skip_group_check=True,
                    )
            rst2 = ep.tile([P, EPB, P], FP32, tag="rst2")
            nc.vector.tensor_tensor(
                rst2[:, 0:nb],
                ffps[:, 0 : nb * P].rearrange("p (s f) -> p s f", f=P),
                ln1[:, 0:nb],
                op=OP.add,
            )
            nc.vector.tensor_tensor(
                rst2[:, 0:nb],
                rst2[:, 0:nb],
                b2rep[:].rearrange("p (o f) -> p o f", o=1).to_broadcast([P, nb, P]),
                op=OP.add,
            )
            ln2 = layernorm(ep, rst2, nb)
            nc.sync.dma_start(
                t["out"][:].rearrange("(s p) f -> p s f", p=P)[:, b0 : b0 + nb, :],
                ln2[:, 0:nb],
            )


def _build(meta, cfg: GATCfg):
    c = cfg
    nc = bacc.Bacc("TRN2", target_bir_lowering=False, debug=False, num_devices=c.n_cores)
    t = {}

    def inp(name, shape, dtype):
        t[name] = nc.dram_tensor(name, shape, dtype, kind="ExternalInput").ap()

    inp("feat16", [c.npad, c.feats], BF16)
    inp("feat16_loc", [c.local_pad, c.feats], BF16)
    inp("feat32_loc", [c.local_pad, c.feats], FP32)
    inp("wq", [c.feats, c.feats], BF16)
    inp("wkv", [c.feats, 2 * c.feats], BF16)
    inp("w1", [c.feats, c.dff], BF16)
    inp("w2a", [P, c.dff // P, c.feats], BF16)
    inp("w2b", [P, c.dff // P, c.feats], BF16)
    inp("b1t", [P, c.dff // P], FP32)
    inp("b2rep", [P, c.feats], FP32)
    inp("grep", [P, c.feats], FP32)
    inp("brep", [P, c.feats], FP32)
    inp("ident", [P, P], BF16)
    inp("kv_idx_lo", [P, max(meta["tot_lo"] // 16, 1)], I16)
    inp("kv_idx_hi", [P, max(meta["tot_hi"] // 16, 1)], I16)
    inp("S_all", [P, meta["tot_cols"]], BF16)
    inp("ST_all", [P, meta["tot_cols"]], BF16)
    t["out"] = nc.dram_tensor(
        "out", [c.local_pad, c.feats], FP32, kind="ExternalOutput"
    ).ap()
    if c.debug:
        t["dbg_ftden"] = nc.dram_tensor(
            "dbg_ftden", [P, c.nblk, 136], FP32, kind="ExternalOutput"
        ).ap()

    with tile.TileContext(nc) as tc:
        _emit(tc, t, meta, cfg)
    nc.compile()
    return nc


def _in_maps(meta, streams, shared, cfg: GATCfg):
    maps = []
    for ci in range(cfg.n_cores):
        m = dict(shared)
        st = streams[ci]
        feat32_loc = st["feat32_loc"]
        m["feat16_loc"] = feat32_loc.astype(bf16)
        m["feat32_loc"] = feat32_loc
        m["kv_idx_lo"] = (
            st["kv_idx_lo"]
            if meta["tot_lo"]
            else np.zeros((P, 1), np.int16)
        )
        m["kv_idx_hi"] = (
            st["kv_idx_hi"]
            if meta["tot_hi"]
            else np.zeros((P, 1), np.int16)
        )
        m["S_all"] = st["S_all"]
        m["ST_all"] = st["ST_all"]
        maps.append(m)
    return maps


_CACHE = {}


def kernel(**inputs) -> np.ndarray:
    cfg = GATCfg()
    meta, streams, shared = _prep(inputs, cfg)
    key = "real"
    if key not in _CACHE:
        _CACHE[key] = _build(meta, cfg)
    nc = _CACHE[key]
    maps = _in_maps(meta, streams, shared, cfg)
    res = run_bass_kernel_spmd(nc, maps, core_ids=list(range(cfg.n_cores)))
    out = np.empty((cfg.n_nodes, cfg.feats), np.float32)
    for ci in range(cfg.n_cores):
        out[ci * cfg.npc : (ci + 1) * cfg.npc] = res.results[ci]["out"][: cfg.npc]
    return out
